# revision 7
# baseline (speedup 1.0000x reference)
"""Trainium2 Bass kernel: multi-scale masked average-pool descriptors.

Computes, per batch element b and scribble i:
    d_l[b,i,c] = mean over {pixels where resize(scribble)[b,i,y,x] > 0.5} of feat_l[b,c,y,x]
    out[b,i,c] = (d_0 + d_1 + d_2) / 3

Key facts exploited:
  * jax.image.resize(bilinear, antialias=False) at scales 4/8/16 reduces to an
    exact 2x2 average at stride k with offset o (k,o) = (4,1)/(8,3)/(16,7):
    sr = 0.25*((a+c)+(b+d)) bit-exactly.  So mask == ((a+c)+(b+d)) > 2.0 with the
    same fp32 association -> masks match the reference bit-exactly.
  * The masked sum is a matmul over pixels: ssum[i,c] = sum_s maskT[s,i]*fmap[c,s].
    Pixel rows y sit on SBUF partitions (the contraction dim K); we iterate over
    pixel columns x with one matmul per x (lhsT = mask column [K,16],
    rhs = channel slice at that x), so fmap is consumed in its native
    [C,h,w] layout via strided DMA -- no transposes.
  * The kernel is HBM/DMA-queue bound: 36.7 MB/core, ~20.5 GB/s per DMA queue
    fair share (measured ~328 GB/s/core cap).  Feature DMAs load the FULL x
    width per call so every descriptor is one whole x-run (512B for feat0,
    3x fewer/bigger than x-chunked loads).  Overlap comes from CHANNEL
    chunking: feat0 streams as 8 c-eighths, each trailed by a series of 128
    N=32 matmuls (~6us, about one eighth's DMA time).
  * DMA queues are FIFO and all descriptor-generators push into the same 16
    queues, so anything pushed early lands early.  Masks gate every matmul,
    so ALL scribble packs are pushed first, and every feature stream's first
    dma_start is gated behind its level's mask tile via a 1-element seed
    write (WAW dep); later chunks are paced by their pool's WAR recycling.
    This keeps the queue FIFOs scribbles-first, features-after -- masks are
    ready just as the first feature chunks land.
  * cnt[i] (mask population count) comes from a [P,16]x[P,1] matmul against ones.
  * The empty-mask fallback is handled on the host (it never triggers for
    non-degenerate inputs; P(empty mask) <= 2^-1024).

Sharding: pure data-parallel over batch B=8 across the 8 NeuronCores.
"""

import numpy as np

_B = 8
_I = 16
_C = 256

# level config by level index: (h, k, off, ipack)
#   h: level size; k: resize stride; off: first-row offset;
#   ipack: scribble images packed per [128, ...] tile
_LEVELS = {
    0: (128, 4, 1, 2),
    1: (64, 8, 3, 2),
    2: (32, 16, 7, 4),
}


def _build_nc():
    import concourse.bacc as bacc
    import concourse.tile as tile
    from concourse import mybir

    f32 = mybir.dt.float32
    f32r = mybir.dt.float32r
    gt = mybir.AluOpType.is_gt
    X = mybir.AxisListType.X

    nc = bacc.Bacc("TRN2", target_bir_lowering=False, debug=False)

    feats = {
        0: nc.dram_tensor("feat0", [_C, 128, 128], f32r, kind="ExternalInput"),
        1: nc.dram_tensor("feat1", [_C, 64, 64], f32r, kind="ExternalInput"),
        2: nc.dram_tensor("feat2", [_C, 32, 32], f32r, kind="ExternalInput"),
    }
    scr = nc.dram_tensor("scribbles", [_I, 512, 512], f32, kind="ExternalInput")
    out_d = nc.dram_tensor("out", [_I, 3 * (_C + 1)], f32, kind="ExternalOutput")

    with tile.TileContext(nc) as tc:
        with (
            tc.tile_pool(name="singles", bufs=1) as singles,
            tc.tile_pool(name="scrib", bufs=4) as scrib,
            tc.tile_pool(name="vtmp", bufs=2) as vtmp,
            tc.tile_pool(name="srtmp", bufs=2) as srtmp,
            tc.tile_pool(name="mtmp", bufs=3) as mtmpp,
            tc.tile_pool(name="f0pool", bufs=5) as f0pool,
            tc.tile_pool(name="f1pool", bufs=2) as f1pool,
            tc.tile_pool(name="f2pool", bufs=2) as f2pool,
            tc.tile_pool(name="psum", bufs=2, space="PSUM") as psum,
        ):
            ones = singles.tile([128, 1], f32, tag="ones")
            nc.vector.memset(ones[:], 1.0)
            stag = singles.tile([_I, 3 * (_C + 1)], f32, tag="stag")

            def make_masks(li):
                """Scribble loads (sync/gpsimd rings) + DVE resize -> mask tile."""
                h, k, off, ipack = _LEVELS[li]
                w = h
                m = singles.tile([h, _I, w], f32r, tag=f"m{li}", name=f"m{li}")
                for t in range(_I // ipack):
                    i0 = t * ipack
                    # rows (k*y+off, k*y+off+1) are adjacent -> merged 4KiB runs
                    seng = nc.sync if t % 2 == 0 else nc.gpsimd
                    if li == 0:
                        # partitions = y(128); free = (i-pair, row-pair * x)
                        st = scrib.tile([128, ipack, 1024], f32, tag="st", name="st")
                        seng.dma_start(
                            out=st[:],
                            in_=scr[i0 : i0 + ipack]
                            .rearrange("i (y k) x -> y i k x", k=k)[
                                :, :, off : off + 2, :
                            ]
                            .rearrange("y i k x -> y i (k x)"),
                        )
                        for il in range(ipack):
                            v = vtmp.tile([128, 512], f32, tag="v", name="v")
                            nc.vector.tensor_add(
                                v[:], st[:, il, 0:512], st[:, il, 512:1024]
                            )
                            vk = v[:].rearrange("p (x k) -> p x k", k=k)
                            sr = srtmp.tile([128, w], f32, tag="sr", name="sr")
                            nc.vector.tensor_add(
                                sr[:], vk[:, :, off], vk[:, :, off + 1]
                            )
                            nc.vector.tensor_scalar(
                                out=m[:, i0 + il, :], in0=sr[:], scalar1=2.0,
                                scalar2=None, op0=gt,
                            )
                    else:
                        # partitions = (i-sub, y); one mask tile per pack,
                        # repacked per-image into m via tiny SBUF->SBUF DMAs
                        st = scrib.tile([128, 1, 1024], f32, tag="st", name="st")
                        seng.dma_start(
                            out=st[:, 0, :].rearrange("p (k x) -> p k x", k=2),
                            in_=scr[i0 : i0 + ipack].rearrange(
                                "i (y k) x -> i y k x", k=k
                            )[:, :, off : off + 2, :],
                        )
                        v = vtmp.tile([128, 512], f32, tag="v", name="v")
                        nc.vector.tensor_add(v[:], st[:, 0, 0:512], st[:, 0, 512:1024])
                        vk = v[:].rearrange("p (x k) -> p x k", k=k)
                        sr = srtmp.tile([128, w], f32, tag="sr", name="sr")
                        nc.vector.tensor_add(sr[:], vk[:, :, off], vk[:, :, off + 1])
                        mt = mtmpp.tile([128, w], f32r, tag="mt", name="mt")
                        nc.vector.tensor_scalar(
                            out=mt[:], in0=sr[:], scalar1=2.0, scalar2=None, op0=gt
                        )
                        for ii in range(ipack):
                            nc.scalar.dma_start(
                                out=m[:, i0 + ii, :],
                                in_=mt[ii * h : (ii + 1) * h, :],
                            )
                return m

            def seed(f, m):
                """1-element write into f from m: the following DMA into f
                (whole-tile WAW) then pushes its descriptors only after the
                mask is done, keeping the queue FIFOs scribbles-first."""
                nc.vector.tensor_copy(f[0:1, 0:1, 0:1], m[0:1, 0:1, 0:1])

            def mm_series(m, f, acc, c0, cg, w):
                """One matmul per pixel column x over channel slice [c0, c0+cg)."""
                for xl in range(w):
                    nc.tensor.matmul(
                        acc[:, c0 : c0 + cg],
                        m[:, :, xl],
                        f[:, :, xl],
                        start=(xl == 0),
                        stop=(xl == w - 1),
                    )

            def finish_level(li, m, acc, slot):
                h = _LEVELS[li][0]
                r = singles.tile([h, _I], f32, tag=f"r{li}", name=f"r{li}")
                nc.vector.reduce_sum(out=r[:], in_=m[:].bitcast(f32), axis=X)
                cntp = psum.tile([_I, 1], f32, tag="cntp", name="cntp")
                nc.tensor.matmul(cntp[:], r[:], ones[:h, :], start=True, stop=True)
                base = slot * (_C + 1)
                nc.vector.tensor_copy(stag[:, base : base + _C], acc[:])
                nc.vector.tensor_copy(stag[:, base + _C : base + _C + 1], cntp[:])

            # ---- emission ----------------------------------------------------
            # all scribble packs are pushed first (masks gate every matmul);
            # each feature stream's first pool generation is seeded so its
            # descriptors queue up only once its mask is ready.
            m1 = make_masks(1)
            m0 = make_masks(0)

            acc1 = psum.tile([_I, _C], f32, tag="acc1", name="acc1")
            for q in range(4):
                f1q = f1pool.tile([64, 64, 64], f32r, tag="f1q", name="f1q")
                if q < 2:
                    seed(f1q, m1)
                nc.scalar.dma_start(
                    out=f1q[:],
                    in_=feats[1][q * 64 : (q + 1) * 64].rearrange("c y x -> y c x"),
                )
                mm_series(m1, f1q, acc1, q * 64, 64, 64)
            finish_level(1, m1, acc1, 1)

            acc0 = psum.tile([_I, _C], f32, tag="acc0", name="acc0")
            for e in range(6):
                f0e = f0pool.tile([128, 32, 128], f32r, tag="f0e", name="f0e")
                if e < 5:
                    seed(f0e, m0)
                nc.scalar.dma_start(
                    out=f0e[:],
                    in_=feats[0][e * 32 : (e + 1) * 32].rearrange("c y x -> y c x"),
                )
                mm_series(m0, f0e, acc0, e * 32, 32, 128)

            # L2 masks + features mid-stream
            m2 = make_masks(2)
            acc2 = psum.tile([_I, _C], f32, tag="acc2", name="acc2")
            for hh in range(2):
                f2h = f2pool.tile([32, 128, 32], f32r, tag="f2h", name="f2h")
                if hh < 2:
                    seed(f2h, m2)
                nc.scalar.dma_start(
                    out=f2h[:],
                    in_=feats[2][hh * 128 : (hh + 1) * 128].rearrange("c y x -> y c x"),
                )
                mm_series(m2, f2h, acc2, hh * 128, 128, 32)
            finish_level(2, m2, acc2, 2)

            # feat0's last two eighths: e6 on gpsimd, e7 split sync+scalar so
            # all three DGEs push the final descriptors.
            f0e6 = f0pool.tile([128, 32, 128], f32r, tag="f0e", name="f0e6")
            nc.gpsimd.dma_start(
                out=f0e6[:], in_=feats[0][192:224].rearrange("c y x -> y c x")
            )
            mm_series(m0, f0e6, acc0, 192, 32, 128)
            f0e7 = f0pool.tile([128, 32, 128], f32r, tag="f0e", name="f0e7")
            nc.sync.dma_start(
                out=f0e7[:, 0:16, :],
                in_=feats[0][224:240].rearrange("c y x -> y c x"),
            )
            nc.scalar.dma_start(
                out=f0e7[:, 16:32, :],
                in_=feats[0][240:256].rearrange("c y x -> y c x"),
            )
            mm_series(m0, f0e7, acc0, 224, 32, 128)
            finish_level(0, m0, acc0, 0)

            nc.gpsimd.dma_start(out=out_d[:], in_=stag[:])

    nc.compile()
    return nc


def _host_fallback(scr_bi, fmap_b, h, k, off):
    """Feature at argmax of the soft mask; only used when a mask is empty."""
    V = scr_bi[off::k, :][:h].astype(np.float32) + scr_bi[off + 1 :: k, :][:h]
    sr4 = V[:, off::k][:, :h] + V[:, off + 1 :: k][:, :h]
    idx = int(np.argmax(np.float32(0.25) * sr4))
    y, x = divmod(idx, h)
    return fmap_b[:, y, x]


def kernel(feat0, feat1, feat2, scribbles):
    import sys

    for p in ("/opt/trn_rl_repo", "/opt/pypackages"):
        if p not in sys.path:
            sys.path.append(p)
    from concourse.bass_utils import run_bass_kernel_spmd

    feat0 = np.asarray(feat0, dtype=np.float32)
    feat1 = np.asarray(feat1, dtype=np.float32)
    feat2 = np.asarray(feat2, dtype=np.float32)
    scribbles = np.asarray(scribbles, dtype=np.float32)

    nc = _build_nc()
    in_maps = [
        {
            "feat0": np.ascontiguousarray(feat0[b]),
            "feat1": np.ascontiguousarray(feat1[b]),
            "feat2": np.ascontiguousarray(feat2[b]),
            "scribbles": np.ascontiguousarray(scribbles[b]),
        }
        for b in range(_B)
    ]
    res = run_bass_kernel_spmd(nc, in_maps, core_ids=list(range(_B)))
    raw = np.stack([res.results[b]["out"] for b in range(_B)])  # [B, I, 3*257]
    raw = raw.reshape(_B, _I, 3, _C + 1)
    ssum = raw[..., :_C].astype(np.float32)  # [B, I, 3, C]
    cnt = raw[..., _C].astype(np.float32)  # [B, I, 3]

    mean = ssum / np.maximum(cnt, np.float32(1.0))[..., None]

    if (cnt == 0).any():  # never for non-degenerate inputs
        fm = [feat0, feat1, feat2]
        for b, i, li in zip(*np.nonzero(cnt == 0)):
            h, k, off, _ = _LEVELS[li]
            mean[b, i, li] = _host_fallback(scribbles[b, i], fm[li][b], h, k, off)

    out = (mean[:, :, 0] + mean[:, :, 1] + mean[:, :, 2]) / np.float32(3.0)
    return out.astype(np.float32)


# revision 8
# speedup vs baseline: 1.0387x; 1.0387x over previous
"""Trainium2 Bass kernel: multi-scale masked average-pool descriptors.

Computes, per batch element b and scribble i:
    d_l[b,i,c] = mean over {pixels where resize(scribble)[b,i,y,x] > 0.5} of feat_l[b,c,y,x]
    out[b,i,c] = (d_0 + d_1 + d_2) / 3

Key facts exploited:
  * jax.image.resize(bilinear, antialias=False) at scales 4/8/16 reduces to an
    exact 2x2 average at stride k with offset o (k,o) = (4,1)/(8,3)/(16,7):
    sr = 0.25*((a+c)+(b+d)) bit-exactly.  So mask == ((a+c)+(b+d)) > 2.0 with the
    same fp32 association -> masks match the reference bit-exactly.
  * The masked sum is a matmul over pixels: ssum[i,c] = sum_s maskT[s,i]*fmap[c,s].
    Pixel rows y sit on SBUF partitions (the contraction dim K); we iterate over
    pixel columns x with one matmul per x (lhsT = mask column [K,16],
    rhs = channel slice at that x), so fmap is consumed in its native
    [C,h,w] layout via strided DMA -- no transposes.
  * The kernel is HBM/DMA-queue bound: 36.7 MB/core, ~20.5 GB/s per DMA queue
    fair share (measured ~328 GB/s/core cap).  Feature DMAs load the FULL x
    width per call so every descriptor is one whole x-run (512B for feat0,
    3x fewer/bigger than x-chunked loads).  Overlap comes from CHANNEL
    chunking: feat0 streams as 8 c-eighths, each trailed by a series of 128
    N=32 matmuls (~6us, about one eighth's DMA time).
  * DMA queues are FIFO and all descriptor-generators push into the same 16
    queues, so anything pushed early lands early.  Masks gate every matmul,
    so ALL scribble packs are pushed first, and every feature stream's first
    dma_start is gated behind its level's mask tile via a 1-element seed
    write (WAW dep); later chunks are paced by their pool's WAR recycling.
    This keeps the queue FIFOs scribbles-first, features-after -- masks are
    ready just as the first feature chunks land.
  * cnt[i] (mask population count) comes from a [P,16]x[P,1] matmul against ones.
  * The empty-mask fallback is handled on the host (it never triggers for
    non-degenerate inputs; P(empty mask) <= 2^-1024).

Sharding: pure data-parallel over batch B=8 across the 8 NeuronCores.
"""

import numpy as np

_B = 8
_I = 16
_C = 256

# level config by level index: (h, k, off, ipack)
#   h: level size; k: resize stride; off: first-row offset;
#   ipack: scribble images packed per [128, ...] tile
_LEVELS = {
    0: (128, 4, 1, 2),
    1: (64, 8, 3, 2),
    2: (32, 16, 7, 4),
}


def _build_nc():
    import concourse.bacc as bacc
    import concourse.tile as tile
    from concourse import mybir

    f32 = mybir.dt.float32
    f32r = mybir.dt.float32r
    gt = mybir.AluOpType.is_gt
    X = mybir.AxisListType.X

    nc = bacc.Bacc("TRN2", target_bir_lowering=False, debug=False)

    feats = {
        0: nc.dram_tensor("feat0", [_C, 128, 128], f32r, kind="ExternalInput"),
        1: nc.dram_tensor("feat1", [_C, 64, 64], f32r, kind="ExternalInput"),
        2: nc.dram_tensor("feat2", [_C, 32, 32], f32r, kind="ExternalInput"),
    }
    scr = nc.dram_tensor("scribbles", [_I, 512, 512], f32, kind="ExternalInput")
    out_d = nc.dram_tensor("out", [_I, 3 * (_C + 1)], f32, kind="ExternalOutput")

    with tile.TileContext(nc) as tc:
        with (
            tc.tile_pool(name="singles", bufs=1) as singles,
            tc.tile_pool(name="scrib", bufs=4) as scrib,
            tc.tile_pool(name="vtmp", bufs=2) as vtmp,
            tc.tile_pool(name="srtmp", bufs=2) as srtmp,
            tc.tile_pool(name="mtmp", bufs=3) as mtmpp,
            tc.tile_pool(name="f0pool", bufs=5) as f0pool,
            tc.tile_pool(name="f1pool", bufs=2) as f1pool,
            tc.tile_pool(name="f2pool", bufs=2) as f2pool,
            tc.tile_pool(name="psum", bufs=2, space="PSUM") as psum,
        ):
            ones = singles.tile([128, 1], f32, tag="ones")
            nc.vector.memset(ones[:], 1.0)
            stag = singles.tile([_I, 3 * (_C + 1)], f32, tag="stag")

            def make_masks(li):
                """Scribble loads (sync/gpsimd rings) + DVE resize -> mask tile."""
                h, k, off, ipack = _LEVELS[li]
                w = h
                m = singles.tile([h, _I, w], f32r, tag=f"m{li}", name=f"m{li}")
                for t in range(_I // ipack):
                    i0 = t * ipack
                    # rows (k*y+off, k*y+off+1) are adjacent -> merged 4KiB runs
                    seng = nc.sync
                    if li == 0:
                        # partitions = y(128); free = (i-pair, row-pair * x)
                        st = scrib.tile([128, ipack, 1024], f32, tag="st", name="st")
                        seng.dma_start(
                            out=st[:],
                            in_=scr[i0 : i0 + ipack]
                            .rearrange("i (y k) x -> y i k x", k=k)[
                                :, :, off : off + 2, :
                            ]
                            .rearrange("y i k x -> y i (k x)"),
                        )
                        for il in range(ipack):
                            v = vtmp.tile([128, 512], f32, tag="v", name="v")
                            nc.vector.tensor_add(
                                v[:], st[:, il, 0:512], st[:, il, 512:1024]
                            )
                            vk = v[:].rearrange("p (x k) -> p x k", k=k)
                            sr = srtmp.tile([128, w], f32, tag="sr", name="sr")
                            nc.vector.tensor_add(
                                sr[:], vk[:, :, off], vk[:, :, off + 1]
                            )
                            nc.vector.tensor_scalar(
                                out=m[:, i0 + il, :], in0=sr[:], scalar1=2.0,
                                scalar2=None, op0=gt,
                            )
                    else:
                        # partitions = (i-sub, y); one mask tile per pack,
                        # repacked per-image into m via tiny SBUF->SBUF DMAs
                        st = scrib.tile([128, 1, 1024], f32, tag="st", name="st")
                        seng.dma_start(
                            out=st[:, 0, :].rearrange("p (k x) -> p k x", k=2),
                            in_=scr[i0 : i0 + ipack].rearrange(
                                "i (y k) x -> i y k x", k=k
                            )[:, :, off : off + 2, :],
                        )
                        v = vtmp.tile([128, 512], f32, tag="v", name="v")
                        nc.vector.tensor_add(v[:], st[:, 0, 0:512], st[:, 0, 512:1024])
                        vk = v[:].rearrange("p (x k) -> p x k", k=k)
                        sr = srtmp.tile([128, w], f32, tag="sr", name="sr")
                        nc.vector.tensor_add(sr[:], vk[:, :, off], vk[:, :, off + 1])
                        mt = mtmpp.tile([128, w], f32r, tag="mt", name="mt")
                        nc.vector.tensor_scalar(
                            out=mt[:], in0=sr[:], scalar1=2.0, scalar2=None, op0=gt
                        )
                        for ii in range(ipack):
                            nc.scalar.dma_start(
                                out=m[:, i0 + ii, :],
                                in_=mt[ii * h : (ii + 1) * h, :],
                            )
                return m

            def seed(f, m):
                """1-element write into f from m: the following DMA into f
                (whole-tile WAW) then pushes its descriptors only after the
                mask is done, keeping the queue FIFOs scribbles-first."""
                nc.vector.tensor_copy(f[0:1, 0:1, 0:1], m[0:1, 0:1, 0:1])

            def mm_series(m, f, acc, c0, cg, w):
                """One matmul per pixel column x over channel slice [c0, c0+cg)."""
                for xl in range(w):
                    nc.tensor.matmul(
                        acc[:, c0 : c0 + cg],
                        m[:, :, xl],
                        f[:, :, xl],
                        start=(xl == 0),
                        stop=(xl == w - 1),
                    )

            def finish_level(li, m, acc, slot):
                h = _LEVELS[li][0]
                r = singles.tile([h, _I], f32, tag=f"r{li}", name=f"r{li}")
                nc.vector.reduce_sum(out=r[:], in_=m[:].bitcast(f32), axis=X)
                cntp = psum.tile([_I, 1], f32, tag="cntp", name="cntp")
                nc.tensor.matmul(cntp[:], r[:], ones[:h, :], start=True, stop=True)
                base = slot * (_C + 1)
                nc.vector.tensor_copy(stag[:, base : base + _C], acc[:])
                nc.vector.tensor_copy(stag[:, base + _C : base + _C + 1], cntp[:])

            # ---- emission ----------------------------------------------------
            # all scribble packs are pushed first (masks gate every matmul);
            # each feature stream's first pool generation is seeded so its
            # descriptors queue up only once its mask is ready.
            m1 = make_masks(1)
            m0 = make_masks(0)

            acc1 = psum.tile([_I, _C], f32, tag="acc1", name="acc1")
            for q in range(4):
                f1q = f1pool.tile([64, 64, 64], f32r, tag="f1q", name="f1q")
                if q < 2:
                    seed(f1q, m1)
                nc.scalar.dma_start(
                    out=f1q[:],
                    in_=feats[1][q * 64 : (q + 1) * 64].rearrange("c y x -> y c x"),
                )
                mm_series(m1, f1q, acc1, q * 64, 64, 64)
            finish_level(1, m1, acc1, 1)

            acc0 = psum.tile([_I, _C], f32, tag="acc0", name="acc0")
            for e in range(6):
                f0e = f0pool.tile([128, 32, 128], f32r, tag="f0e", name="f0e")
                if e < 5:
                    seed(f0e, m0)
                nc.scalar.dma_start(
                    out=f0e[:],
                    in_=feats[0][e * 32 : (e + 1) * 32].rearrange("c y x -> y c x"),
                )
                mm_series(m0, f0e, acc0, e * 32, 32, 128)

            # L2 masks + features mid-stream
            m2 = make_masks(2)
            acc2 = psum.tile([_I, _C], f32, tag="acc2", name="acc2")
            for hh in range(2):
                f2h = f2pool.tile([32, 128, 32], f32r, tag="f2h", name="f2h")
                if hh < 2:
                    seed(f2h, m2)
                nc.scalar.dma_start(
                    out=f2h[:],
                    in_=feats[2][hh * 128 : (hh + 1) * 128].rearrange("c y x -> y c x"),
                )
                mm_series(m2, f2h, acc2, hh * 128, 128, 32)
            finish_level(2, m2, acc2, 2)

            # feat0's last two eighths: e6 on gpsimd, e7 split sync+scalar so
            # all three DGEs push the final descriptors.
            f0e6 = f0pool.tile([128, 32, 128], f32r, tag="f0e", name="f0e6")
            nc.gpsimd.dma_start(
                out=f0e6[:], in_=feats[0][192:224].rearrange("c y x -> y c x")
            )
            mm_series(m0, f0e6, acc0, 192, 32, 128)
            f0e7 = f0pool.tile([128, 32, 128], f32r, tag="f0e", name="f0e7")
            nc.sync.dma_start(
                out=f0e7[:, 0:16, :],
                in_=feats[0][224:240].rearrange("c y x -> y c x"),
            )
            nc.scalar.dma_start(
                out=f0e7[:, 16:32, :],
                in_=feats[0][240:256].rearrange("c y x -> y c x"),
            )
            mm_series(m0, f0e7, acc0, 224, 32, 128)
            finish_level(0, m0, acc0, 0)

            nc.gpsimd.dma_start(out=out_d[:], in_=stag[:])

    nc.compile()
    return nc


def _host_fallback(scr_bi, fmap_b, h, k, off):
    """Feature at argmax of the soft mask; only used when a mask is empty."""
    V = scr_bi[off::k, :][:h].astype(np.float32) + scr_bi[off + 1 :: k, :][:h]
    sr4 = V[:, off::k][:, :h] + V[:, off + 1 :: k][:, :h]
    idx = int(np.argmax(np.float32(0.25) * sr4))
    y, x = divmod(idx, h)
    return fmap_b[:, y, x]


def kernel(feat0, feat1, feat2, scribbles):
    import sys

    for p in ("/opt/trn_rl_repo", "/opt/pypackages"):
        if p not in sys.path:
            sys.path.append(p)
    from concourse.bass_utils import run_bass_kernel_spmd

    feat0 = np.asarray(feat0, dtype=np.float32)
    feat1 = np.asarray(feat1, dtype=np.float32)
    feat2 = np.asarray(feat2, dtype=np.float32)
    scribbles = np.asarray(scribbles, dtype=np.float32)

    nc = _build_nc()
    in_maps = [
        {
            "feat0": np.ascontiguousarray(feat0[b]),
            "feat1": np.ascontiguousarray(feat1[b]),
            "feat2": np.ascontiguousarray(feat2[b]),
            "scribbles": np.ascontiguousarray(scribbles[b]),
        }
        for b in range(_B)
    ]
    res = run_bass_kernel_spmd(nc, in_maps, core_ids=list(range(_B)))
    raw = np.stack([res.results[b]["out"] for b in range(_B)])  # [B, I, 3*257]
    raw = raw.reshape(_B, _I, 3, _C + 1)
    ssum = raw[..., :_C].astype(np.float32)  # [B, I, 3, C]
    cnt = raw[..., _C].astype(np.float32)  # [B, I, 3]

    mean = ssum / np.maximum(cnt, np.float32(1.0))[..., None]

    if (cnt == 0).any():  # never for non-degenerate inputs
        fm = [feat0, feat1, feat2]
        for b, i, li in zip(*np.nonzero(cnt == 0)):
            h, k, off, _ = _LEVELS[li]
            mean[b, i, li] = _host_fallback(scribbles[b, i], fm[li][b], h, k, off)

    out = (mean[:, :, 0] + mean[:, :, 1] + mean[:, :, 2]) / np.float32(3.0)
    return out.astype(np.float32)
